# revision 1
# baseline (speedup 1.0000x reference)
"""GAT-style edge-affinity layer (nn_Decode_Cora) on 8 Trainium2 NeuronCores.

Sharding: each core owns a 512-node slice of the graph. It projects its own
nodes (g = vert @ W), computes attention-numerator/denominator partial sums
over its 512 source nodes j for ALL 4096 destinations i, and a ReduceScatter
(in destination-major layout) hands each core its 512 output rows for the
final divide + ELU.

Math: softmax rows are invariant to per-row scaling, so
    p[i,j] = mask * exp(lrelu(sl_i + sr_j))
           ∝ mask * exp(0.2*sr_j) * exp(relu(0.8*(sl_i+sr_j)))
           = mask * max(exp(0.8*sl_i + sr_j), exp(0.2*sr_j))
which needs only one ACT exp (per-partition bias sr_j) and one fused
scalar_tensor_tensor (max with exp(0.2*sr_j), then multiply by mask).
"""

import sys

for _p in ("/opt/trn_rl_repo",):
    if _p not in sys.path:
        sys.path.append(_p)

import numpy as np
import ml_dtypes

import concourse.bass as bass
import concourse.bacc as bacc
import concourse.mybir as mybir
import concourse.tile as tile
from concourse.masks import make_identity

f32 = mybir.dt.float32
f16 = mybir.dt.float16

N = 4096          # nodes
F = 1433          # input features
FP = 1536         # padded features (12 * 128)
KT = FP // 128    # 12 contraction tiles
H = 8             # heads
DH = 8            # per-head dim
HD = H * DH       # 64
NC = 8            # cores
NL = N // NC      # 512 nodes per core
NCH = NL // 128   # 4 local j-chunks
NIS = N // 512    # 8 destination column slices
LRELU = 0.2

_STATE = {}


def _build_program(repeat=1, null=False, nocc=False, debug=False, variant='b'):
    nc = bacc.Bacc("TRN2", target_bir_lowering=False, debug=False, num_devices=NC)

    vt = nc.dram_tensor("vt", [FP, NL], f16, kind="ExternalInput")
    vtl = nc.dram_tensor("vtl", [FP, NL], f16, kind="ExternalInput")
    wp = nc.dram_tensor("wp", [FP, HD], f16, kind="ExternalInput")
    wpl = nc.dram_tensor("wpl", [FP, HD], f16, kind="ExternalInput")
    al8 = nc.dram_tensor("al8", [128, H], f32, kind="ExternalInput")
    ar = nc.dram_tensor("ar", [128, H], f32, kind="ExternalInput")
    mskt = nc.dram_tensor("mskt", [NL, N], f16, kind="ExternalInput")
    out = nc.dram_tensor("out", [NL, HD], f32, kind="ExternalOutput")

    dbg = {}
    if debug:
        dbg["slg"] = nc.dram_tensor("dslg", [NC * H, NL], f16, kind="ExternalOutput")
        dbg["sr"] = nc.dram_tensor("dsr", [128, NCH * H], f32, kind="ExternalOutput")
        dbg["esr"] = nc.dram_tensor("desr", [128, NCH * H], f32, kind="ExternalOutput")
        dbg["slb0"] = nc.dram_tensor("dslb0", [128, N], f16, kind="ExternalOutput")
        dbg["pm0"] = nc.dram_tensor("dpm0", [128, N], f16, kind="ExternalOutput")
        dbg["numt"] = nc.dram_tensor("dnumt", [N, 72], f32, kind="ExternalOutput")
        dbg["gr"] = nc.dram_tensor("dgr", [128, NCH * 72], f16, kind="ExternalOutput")

    # internal DRAM for collectives
    sl_loc = nc.dram_tensor("sl_loc", [H, NL], f16)
    slg = nc.dram_tensor("slg", [NC * H, NL], f16, addr_space="Shared")
    numt = nc.dram_tensor("numt", [N, 72], f32)
    numt_rs = nc.dram_tensor("numt_rs", [NL, 72], f32)

    if null:
        with tile.TileContext(nc) as tc:
            with tc.tile_pool(name="np0", bufs=1) as p0:
                t0 = p0.tile([128, 64], f16)
                t1 = p0.tile([128, 64], f32)
                for b in range(NL // 128):
                    nc.sync.dma_start(t0[:], vt[128 * b:128 * (b + 1), 0:64])
                    nc.vector.tensor_copy(t1[:], t0[:])
                    nc.sync.dma_start(out[128 * b:128 * (b + 1), :], t1[:])
        nc.compile()
        return nc

    with tile.TileContext(nc) as tc:
        with (
            tc.tile_pool(name="const", bufs=1) as cp,
            tc.tile_pool(name="psum", bufs=8, space="PSUM") as pp,
        ):
            # ---- constants / big resident tiles ----
            w_sb = cp.tile([128, KT, HD], f16)
            nc.sync.dma_start(w_sb[:], wp[:].rearrange("(k p) d -> p k d", p=128))
            wl_sb = cp.tile([128, KT, HD], f16)
            nc.sync.dma_start(wl_sb[:], wpl[:].rearrange("(k p) d -> p k d", p=128))
            al_sb = cp.tile([128, H], f32)
            nc.sync.dma_start(al_sb[:], al8[:])
            ar_sb = cp.tile([128, H], f32)
            nc.sync.dma_start(ar_sb[:], ar[:])
            msk_sb = cp.tile([128, NCH, N], f16)
            gt_sb = cp.tile([128, NL], f32)      # g^T padded to 128 partitions
            sr_sb = cp.tile([128, NCH * H], f32)   # sr per chunk, col 8c+h
            esr_sb = cp.tile([128, NCH * H], f32)  # exp(0.2 sr)
            gr_sb = cp.tile([128, NCH * 72], f16)  # lhsT per chunk/head + ones col
            ntb = None
            if variant == 'b':
                ntb = cp.tile([128, N // 128, 72], f32, name="ntb")

            # ---- phase 1: projection (vt pool closed early to free SBUF) ----
            with tc.tile_pool(name="vtp", bufs=1) as vtp:
                vts, vtls = [], []
                for k in range(KT):
                    vtt = vtp.tile([128, NL], f16, name=f"vt{k}")
                    nc.sync.dma_start(vtt[:], vt[:].rearrange("(k p) n -> k p n", k=KT)[k])
                    vts.append(vtt)
                for k in range(KT):
                    vtt = vtp.tile([128, NL], f16, name=f"vtl{k}")
                    nc.sync.dma_start(vtt[:], vtl[:].rearrange("(k p) n -> k p n", k=KT)[k])
                    vtls.append(vtt)
                nc.vector.memset(gt_sb[64:128, :], 0.0)
                gt_ps = pp.tile([128, 512], f32, tag="bank", name="gt_ps")
                _gt_ops = ([(w_sb, vts)] * KT and
                           [(w_sb, vts, k) for k in range(KT)] +
                           [(w_sb, vtls, k) for k in range(KT)] +
                           [(wl_sb, vts, k) for k in range(KT)])
                for i, (wsrc, vsrc, k) in enumerate(_gt_ops):
                    nc.tensor.matmul(gt_ps[0:HD, :], wsrc[:, k, :], vsrc[k][:],
                                     start=(i == 0), stop=(i == len(_gt_ops) - 1))
                nc.vector.tensor_copy(gt_sb[0:HD, :], gt_ps[0:HD, :])

                # sl8T local: [H, NL] via lhsT=0.8*A_l (padded), rhs=gT
                sl_ps = pp.tile([128, 512], f32, tag="bank", name="sl_ps")
                nc.tensor.matmul(sl_ps[0:H, :], al_sb[:], gt_sb[:], start=True, stop=True)
                sl_sb = cp.tile([H, NL], f16, name="sl_sb")
                nc.vector.tensor_copy(sl_sb[:], sl_ps[0:H, :])
                nc.sync.dma_start(sl_loc[:], sl_sb[:])

                for c in range(NCH):
                    nc.sync.dma_start(
                        msk_sb[:, c, :],
                        mskt[:].rearrange("(c p) i -> c p i", p=128)[c])

                for c in range(NCH):
                    # g for this chunk: [128 nodes, 64]
                    g_ps = pp.tile([128, 512], f32, tag="bank", name="g_ps")
                    _g_ops = ([(vts, w_sb, k) for k in range(KT)] +
                              [(vtls, w_sb, k) for k in range(KT)] +
                              [(vts, wl_sb, k) for k in range(KT)])
                    for i, (vsrc, wsrc, k) in enumerate(_g_ops):
                        nc.tensor.matmul(g_ps[:, 0:HD], vsrc[k][:, 128 * c:128 * (c + 1)],
                                         wsrc[:, k, :], start=(i == 0),
                                         stop=(i == len(_g_ops) - 1))
                    # sr for this chunk
                    sr_ps = pp.tile([128, 512], f32, tag="bank", name="sr_ps")
                    nc.tensor.matmul(sr_ps[:, 0:H], gt_sb[:, 128 * c:128 * (c + 1)],
                                     ar_sb[:], start=True, stop=True)
                    nc.vector.tensor_copy(sr_sb[:, H * c:H * (c + 1)], sr_ps[:, 0:H])
                    nc.scalar.activation(esr_sb[:, H * c:H * (c + 1)], sr_ps[:, 0:H],
                                         mybir.ActivationFunctionType.Exp, scale=0.2)
                    # lhsT tile: [g_h | ones] interleaved, 9 cols per head
                    grc = gr_sb[:, 72 * c:72 * (c + 1)].rearrange("p (h k) -> p h k", k=9)
                    for h in range(H):
                        nc.vector.tensor_copy(grc[:, h, 0:8], g_ps[:, 8 * h:8 * (h + 1)])
                    nc.vector.memset(grc[:, :, 8], 1.0)

            # ---- phase 2: AllGather sl ----
            if nocc:
                nc.sync.dma_start(slg[0:H, :], sl_sb[:])
                for _s in range(1, NC):
                    nc.sync.dma_start(slg[H * _s:H * (_s + 1), :], sl_sb[:])
            else:
                nc.gpsimd.collective_compute(
                    "AllGather", mybir.AluOpType.bypass,
                    replica_groups=[list(range(NC))],
                    ins=[sl_loc[:].opt()], outs=[slg[:].opt()],
                )

            if debug:
                nc.sync.dma_start(dbg["slg"][:], slg[:])
                nc.sync.dma_start(dbg["sr"][:], sr_sb[:])
                nc.sync.dma_start(dbg["esr"][:], esr_sb[:])
                nc.sync.dma_start(dbg["gr"][:], gr_sb[:])

            # ---- phase 3: main attention loop ----
            with (
                tc.tile_pool(name="slbp", bufs=3) as slbp,
                tc.tile_pool(name="tp", bufs=3) as tp,
                tc.tile_pool(name="pmp", bufs=4) as pmp,
                tc.tile_pool(name="nhp", bufs=2) as nhp,
                tc.tile_pool(name="small", bufs=4) as sp,
            ):
              for _rep in range(repeat):
                for h in range(H):
                    slb = slbp.tile([128, N], f16, name="slb")
                    _src = slg[:].rearrange("(s h2) f -> h2 s f", h2=H)[h:h+1]
                    nc.sync.dma_start(
                        slb[:].rearrange("p (s f) -> p s f", s=NIS),
                        _src.to_broadcast([128, NIS, NL]),
                    )
                    # one PSUM bank holds the whole head: [128 dest, 32 blk * 9]
                    bank = pp.tile([128, 512], f32, tag="bank", name="bank")
                    for c in range(NCH):
                        t = tp.tile([128, N], f16, name="texp")
                        nc.scalar.activation(t[:], slb[:], mybir.ActivationFunctionType.Exp,
                                             bias=sr_sb[:, H * c + h:H * c + h + 1])
                        u = tp.tile([128, N], f16, name="umax")
                        nc.vector.tensor_scalar(u[:], t[:],
                                                esr_sb[:, H * c + h:H * c + h + 1],
                                                None, mybir.AluOpType.max)
                        pm = pmp.tile([128, N], f16, name="pm")
                        nc.vector.tensor_tensor(pm[:], u[:], msk_sb[:, c, :],
                                                mybir.AluOpType.mult)
                        rhs = gr_sb[:, 72 * c + 9 * h:72 * c + 9 * (h + 1)]
                        for ib in range(N // 128):
                            nc.tensor.matmul(bank[:, 9 * ib:9 * (ib + 1)],
                                             pm[:, 128 * ib:128 * (ib + 1)], rhs,
                                             start=(c == 0 and ib == 0),
                                             stop=(c == NCH - 1 and ib == N // 128 - 1),
                                             skip_group_check=True)
                    if debug and h == 0 and _rep == 0:
                        nc.sync.dma_start(dbg["slb0"][:], slb[:])
                        nc.sync.dma_start(dbg["pm0"][:], pm[:])
                    if variant == 'b':
                        nc.vector.tensor_copy(
                            ntb[:, :, 9 * h:9 * (h + 1)],
                            bank[:, 0:288].rearrange("p (b k) -> p b k", k=9))
                    else:
                        nh_sb = sp.tile([128, 288], f32, name="nh_sb")
                        nc.vector.tensor_copy(nh_sb[:], bank[:, 0:288])
                        nc.sync.dma_start(
                            numt[:].rearrange("(b p) m -> p b m", p=128)[:, :, 9 * h:9 * (h + 1)],
                            nh_sb[:].rearrange("p (b k) -> p b k", k=9))
              if True:

                if debug:
                    nc.sync.dma_start(dbg["numt"][:], numt[:])

                if variant == 'b':
                    nc.sync.dma_start(
                        numt[:].rearrange("(b p) m -> p b m", p=128), ntb[:])

                # ---- phase 5: ReduceScatter ----
                if nocc:
                    nc.sync.dma_start(numt_rs[:], numt[0:NL, :])
                else:
                    nc.gpsimd.collective_compute(
                        "ReduceScatter", mybir.AluOpType.add,
                        replica_groups=[list(range(NC))],
                        ins=[numt[:].opt()], outs=[numt_rs[:].opt()],
                    )

                # ---- phase 6: divide + ELU ----
                for b in range(NL // 128):
                    nf = sp.tile([128, 72], f32, name="nf")
                    nc.sync.dma_start(nf[:], numt_rs[128 * b:128 * (b + 1), :])
                    nfr = nf.rearrange("p (h k) -> p h k", k=9)
                    rec = sp.tile([128, H], f32, name="rec")
                    nc.vector.reciprocal(rec[:], nfr[:, :, 8])
                    aout = sp.tile([128, HD], f32, name="aout")
                    for h in range(H):
                        nc.vector.tensor_scalar(aout[:, 8 * h:8 * (h + 1)], nfr[:, h, 0:8],
                                                rec[:, h:h + 1], None, mybir.AluOpType.mult)
                    # elu(x) = relu(x) - 1 + exp(min(x, 0))
                    xm = sp.tile([128, HD], f32, name="xm")
                    nc.vector.tensor_scalar(xm[:], aout[:], 0.0, None, mybir.AluOpType.min)
                    ex = sp.tile([128, HD], f32, name="ex")
                    nc.scalar.activation(ex[:], xm[:], mybir.ActivationFunctionType.Exp)
                    r1 = sp.tile([128, HD], f32, name="r1")
                    nc.vector.tensor_scalar(r1[:], aout[:], 0.0, -1.0,
                                            mybir.AluOpType.max, mybir.AluOpType.add)
                    ot = sp.tile([128, HD], f32, name="ot")
                    nc.vector.tensor_tensor(ot[:], ex[:], r1[:], mybir.AluOpType.add)
                    nc.sync.dma_start(out[128 * b:128 * (b + 1), :], ot[:])

    nc.compile()
    return nc


def _prep_inputs(vert, edge, W, a_l, a_r):
    vert = np.asarray(vert, dtype=np.float32)
    edge = np.asarray(edge)
    W = np.asarray(W, dtype=np.float32)
    a_l = np.asarray(a_l, dtype=np.float32)
    a_r = np.asarray(a_r, dtype=np.float32)

    vtp32 = np.zeros((FP, N), dtype=np.float32)
    vtp32[:F] = vert.T
    vtp = vtp32.astype(np.float16)
    vtl = (vtp32 - vtp.astype(np.float32)).astype(np.float16)
    wp32 = np.zeros((FP, HD), dtype=np.float32)
    wp32[:F] = W
    wp = wp32.astype(np.float16)
    wpl = (wp32 - wp.astype(np.float32)).astype(np.float16)

    al8 = np.zeros((128, H), dtype=np.float32)
    ar8 = np.zeros((128, H), dtype=np.float32)
    for h in range(H):
        al8[8 * h:8 * (h + 1), h] = 0.8 * a_l[h]
        ar8[8 * h:8 * (h + 1), h] = a_r[h]

    maskT = (edge != 0).astype(np.float16)  # [i, j] -> transpose below

    in_maps = []
    for c in range(NC):
        sl = slice(512 * c, 512 * (c + 1))
        in_maps.append({
            "vt": np.ascontiguousarray(vtp[:, sl]),
            "vtl": np.ascontiguousarray(vtl[:, sl]),
            "wp": wp,
            "wpl": wpl,
            "al8": al8,
            "ar": ar8,
            "mskt": np.ascontiguousarray(maskT[:, sl].T),
        })
    return in_maps


def _get_runner(repeat=1, null=False, variant='b'):
    """Build (once) and return a callable in_maps -> list of per-core outputs."""
    key = f"runner{repeat}_{null}_{variant}"
    if key in _STATE:
        return _STATE[key]

    nc = _build_program(repeat, null, variant=variant)

    import jax
    from jax.sharding import Mesh, PartitionSpec
    from jax.experimental.shard_map import shard_map
    from concourse import bass2jax
    from concourse.bass2jax import _bass_exec_p, partition_id_tensor

    bass2jax.install_neuronx_cc_hook()

    partition_name = nc.partition_id_tensor.name if nc.partition_id_tensor else None
    in_names, out_names, out_avals, zero_shapes = [], [], [], []
    for alloc in nc.m.functions[0].allocations:
        if not isinstance(alloc, mybir.MemoryLocationSet):
            continue
        name = alloc.memorylocations[0].name
        if alloc.kind == "ExternalInput":
            if name != partition_name:
                in_names.append(name)
        elif alloc.kind == "ExternalOutput":
            shape = tuple(alloc.tensor_shape)
            dtype = mybir.dt.np(alloc.dtype)
            out_names.append(name)
            out_avals.append(jax.core.ShapedArray(shape, dtype))
            zero_shapes.append((shape, dtype))
    n_params = len(in_names)
    n_outs = len(out_avals)
    all_in_names = list(in_names) + list(out_names)
    if partition_name is not None:
        all_in_names.append(partition_name)
    donate = tuple(range(n_params, n_params + n_outs))

    def _body(*args):
        operands = list(args)
        if partition_name is not None:
            operands.append(partition_id_tensor())
        outs = _bass_exec_p.bind(
            *operands,
            out_avals=tuple(out_avals),
            in_names=tuple(all_in_names),
            out_names=tuple(out_names),
            lowering_input_output_aliases=(),
            sim_require_finite=True,
            sim_require_nnan=True,
            nc=nc,
        )
        return tuple(outs)

    devices = jax.devices()[:NC]
    mesh = Mesh(np.asarray(devices), ("core",))
    in_specs = (PartitionSpec("core"),) * (n_params + n_outs)
    out_specs = (PartitionSpec("core"),) * n_outs
    sharded = jax.jit(
        shard_map(_body, mesh=mesh, in_specs=in_specs, out_specs=out_specs,
                  check_rep=False),
        donate_argnums=donate, keep_unused=True,
    )

    def runner(in_maps):
        concat_in = [
            np.concatenate([np.asarray(in_maps[c][nm]) for c in range(NC)], axis=0)
            for nm in in_names
        ]
        concat_zeros = [
            np.zeros((NC * s[0], *s[1:]), dt) for (s, dt) in zero_shapes
        ]
        out_arrs = sharded(*concat_in, *concat_zeros)
        out_arrs = [np.asarray(a) for a in out_arrs]
        return [
            {nm: out_arrs[i].reshape(NC, *out_avals[i].shape)[c]
             for i, nm in enumerate(out_names)}
            for c in range(NC)
        ]

    _STATE[key] = runner
    _STATE[f"internals{repeat}_{null}_{variant}"] = {
        "sharded": sharded, "in_names": in_names, "zero_shapes": zero_shapes,
        "mesh": mesh, "out_names": out_names, "out_avals": out_avals,
    }
    return runner


def kernel(vert, edge, W, a_l, a_r):
    in_maps = _prep_inputs(vert, edge, W, a_l, a_r)
    runner = _get_runner()
    results = runner(in_maps)
    return np.concatenate([results[c]["out"] for c in range(NC)], axis=0)

